# revision 55
# baseline (speedup 1.0000x reference)
"""Multi-head attention (B=4, T=2048, D=1024, H=16, DH=64) on 8 Trainium2 NeuronCores.

Sharding (data + tensor parallel, no collectives):
  core c owns batch b = c//2 and heads [(c%2)*8, (c%2)*8 + 8).
  Each core computes q/k/v projections for its batch over its 8 heads, the
  full attention for those (batch, head) slabs, and a partial output
  projection over its heads' columns of w_out.  The host sums the two
  partial outputs per batch (the only cross-core reduction).

Key optimizations vs a straightforward fp32r implementation (~477us -> ~355us
measured; the major wins):
  - All matmul operands are bf16 (fp32 moving operands stream at 2 cyc/row on
    the PE; bf16 streams at 1 -> 2x matmul throughput).  PSUM stays fp32.
  - The softmax exp (33.5M elements/core, a ScalarE bottleneck at 1
    elem/lane/cycle) alternates between ScalarE (exact exp) and the DVE
    (Schraudolph-style bit-trick exp emitting bf16 bit patterns via a single
    tensor_scalar mult+add with uint16 convert).  The softmax is
    self-normalizing, so the ~2% sawtooth error of the bit-trick on half the
    j-positions washes out to ~1e-2 relative error in the output.
  - Attention runs as 3-slot batches (slot = one (i-block, j-chunk) for a
    head pair): the PE sees runs of 6 QK then 6 AV matmuls -- same-type
    back-to-back matmuls pipeline at the 213ns streaming rate while
    alternating types exposes a ~170ns pipeline fill each transition.  AV
    consumption lags exp by 4+ slots so neither exp engine stalls the PE.
  - q/k projections are interleaved per head-pair with that pair's
    attention; softmax normalization runs per-i-block as AV blocks drain
    (GPSIMD does the multiplies), so the output projection is never gated
    on a serial end-of-kernel normalize.

Device algorithm (per core), everything feature-major ("transposed") so the
contraction dim always lands on SBUF partitions:
  qT = Wq @ x^T            [512, T]   (rows = h_local*64 + d)
  kT = Wk @ x^T            [512, T]
  V  = x @ Wv^T            [T, 512]   (+ a ones column per head => denominator)
  per head h: ST = kT_h^T-contraction  => scores [j, i] in PSUM,
              E = exp(ST/32) (ScalarE fused scale, or DVE bit-trick),
              oT_h' = [V_h | 1]^T @ E  => [65, i] (row 64 = softmax denom),
              oT_h = oT_h[0:64] * (1/denom) broadcast (DRAM-bounce bcast + DVE)
  out_partial = oT^T @ Wo_slice^T  [T, 1024]
"""

import os
import sys

import numpy as np
import ml_dtypes

if "/opt/trn_rl_repo" not in sys.path and os.path.isdir("/opt/trn_rl_repo"):
    sys.path.insert(0, "/opt/trn_rl_repo")

import concourse.bass as bass  # noqa: E402
import concourse.tile as tile  # noqa: E402
from concourse import bacc, mybir  # noqa: E402
from concourse.bass_utils import run_bass_kernel_spmd  # noqa: E402


def bass_AP_bcast(tile_ap, parts, free):
    """Partition-broadcast AP over a DRAM [1, free] tile (step-0 partition)."""
    a = tile_ap[:]
    return bass.AP(tensor=a.tensor, offset=a.offset,
                   ap=[[0, parts]] + [list(x) for x in a.ap[1:]])

F32 = mybir.dt.float32
BF16 = mybir.dt.bfloat16
U16 = mybir.dt.uint16
AF = mybir.ActivationFunctionType
OP = mybir.AluOpType

B, T, D, H, DH = 4, 2048, 1024, 16, 64
HL = H // 2          # heads per core
E = HL * DH          # 512: per-core q (or k, or v) feature width
KO = D // 128        # 8 contraction chunks for the projections
P = 128
SCALE = float(D) ** -0.5  # note: dim**-0.5, faithful to the reference

# Schraudolph exp -> bf16 bits: bf16(exp(SCALE*s)) ~= u16(EXP_A*s+EXP_B).
# A = SCALE*log2(e)*2^7 maps the exponent into the bf16 exponent field;
# B centers at bias 127 with a -5.50/128 minimax offset balancing the
# (1+f)/2^f mantissa-linear approximation, +0.5 assuming truncating convert
# (if the hw rounds instead, the uniform x2^(0.5/128) factor cancels in the
# softmax).
EXP_A = SCALE * 1.4426950408889634 * 128.0
EXP_B = 127.0 * 128.0 - 5.50 + 0.5

# exp engine schedule: per attention slot, cycle through this pattern.
# True = ScalarE exact exp, False = DVE bit-trick.
EXP_PATTERN = (True, False)  # alternate ACT / DVE

_cache = {}
last_results = None


def _emit(ctx, tc, nc, xT, wq, wk, wv, wo, out, t):
    TB = t // 512        # moving-dim blocks for projections
    TC = t // 128        # t chunks (also j chunks)
    JC = t // 128
    IBN = t // 512       # query i-blocks

    xT3 = xT.rearrange("tb p (ko f) -> tb p ko f", ko=KO)
    wq3 = wq.rearrange("p (ko e) -> p ko e", ko=KO)
    wk3 = wk.rearrange("p (ko e) -> p ko e", ko=KO)
    wv3 = wv.rearrange("p (ko e) -> p ko e", ko=KO)
    wo3 = wo.rearrange("(c p) d -> p c d", p=P)

    persist = ctx.enter_context(tc.tile_pool(name="persist", bufs=1))
    # q/k tiles split per (eo, tb) so attention slots dep only on the copies
    # they actually read (tile-granular tracking would gate each pair's first
    # QK on the final projection copy).  eo 0-3: q pairs, 4-7: k pairs.
    qkt = [[persist.tile([P, 512], BF16, tag=f"qkT{eo}_{tb}",
                         name=f"qkT{eo}_{tb}")
            for tb in range(TB)] for eo in range(8)]
    vsb = persist.tile([P, JC, HL, DH + 1], BF16, tag="v")
    ones1 = persist.tile([P, 1], BF16, tag="ones")
    nc.vector.memset(ones1[:], 1.0)
    nc.vector.tensor_copy(vsb[:, :, :, DH], ones1.to_broadcast((P, JC, HL)))
    oT_all = persist.tile([P, 4, t], BF16, tag="oT")

    wp = ctx.enter_context(tc.tile_pool(name="w_p", bufs=1))
    wv_sb = wp.tile([P, KO, E], BF16, tag="wv")
    wq_sb = wp.tile([P, KO, E], BF16, tag="wq")
    wk_sb = wp.tile([P, KO, E], BF16, tag="wk")
    wo_sb = wp.tile([P, 4, D], BF16, tag="wo")
    xts = [wp.tile([P, KO, 512], BF16, tag=f"xt{tb}", name=f"xt{tb}")
           for tb in range(TB)]
    # Inputs split across the two hwdge queues (sync + scalar) so x and the
    # weights stream in parallel; gpsimd queue takes the non-urgent wo.  The
    # first V-proj matmul group needs xt0[ko] + wv[ko] in ko order, so those
    # two stream as ko-pair chunks on separate queues and the projection
    # starts ~2us after the engine-sync preamble instead of waiting for
    # whole-tensor DMAs.
    for c in range(KO // 2):
        nc.sync.dma_start(xts[0][:, 2 * c:2 * c + 2, :],
                          xT3[0][:, 2 * c:2 * c + 2, :])
        (nc.scalar if c < 2 else nc.sync).dma_start(
            wv_sb[:, 2 * c:2 * c + 2, :], wv3[:, 2 * c:2 * c + 2, :])
    nc.gpsimd.dma_start(xts[1][:], xT3[1])
    nc.sync.dma_start(xts[2][:], xT3[2])
    nc.scalar.dma_start(xts[3][:], xT3[3])
    nc.scalar.dma_start(wq_sb[:], wq3)
    nc.scalar.dma_start(wk_sb[:], wk3)
    nc.gpsimd.dma_start(wo_sb[:], wo3)

    # ---- V projection (needed by every pair's AV) ----
    with tc.tile_pool(name="vps", bufs=4, space="PSUM") as vpsp:
        for tci in range(TC):
            xt = xts[tci // 4]
            sub = tci % 4
            ps = vpsp.tile([P, 512], F32, tag="vps")
            for ko in range(KO):
                nc.tensor.matmul(ps[:], xt[:, ko, sub * 128:(sub + 1) * 128],
                                 wv_sb[:, ko, :],
                                 start=(ko == 0), stop=(ko == KO - 1))
            nc.vector.tensor_copy(vsb[:, tci, :, 0:DH],
                                  ps.rearrange("p (h d) -> p h d", d=DH))

    # shared SBUF pools for attention
    asb = ctx.enter_context(tc.tile_pool(name="attn", bufs=1))
    expp = ctx.enter_context(tc.tile_pool(name="expp", bufs=8))
    bcp = ctx.enter_context(tc.tile_pool(name="bc", bufs=1))
    dramp = ctx.enter_context(tc.tile_pool(name="dram", bufs=2, space="DRAM"))

    def normalize(pr, oacc, i0, i1):
        # oT[:, pr, i0:i1] = oacc[0:64, i0:i1] * (1 / denom), denom = row 64.
        # The denom row sits on ONE partition; bounce it through DRAM to
        # reshape to [128, w/128] (reciprocal on 128 lanes), back to DRAM,
        # then DMA-broadcast from DRAM across 64 partitions.  The B-half
        # product needs a partition shift to rows 64-127: SBUF->SBUF DMA.
        # hb=0 and hb=1 chains are independent (distinct tags) and overlap.
        w = i1 - i0
        for hb in range(2):
            dd1 = dramp.tile([1, w], F32, tag=f"dd1{hb}", name=f"dd1{hb}")
            nc.sync.dma_start(dd1[:], oacc[hb][64:65, i0:i1])
            den = bcp.tile([P, w // P], F32, tag=f"den{hb}", name=f"den{hb}")
            nc.sync.dma_start(den[:], dd1.rearrange("o (p f) -> (o p) f", p=P))
            nc.vector.reciprocal(den[:], den[:])
            dd2 = dramp.tile([1, w], F32, tag=f"dd2{hb}", name=f"dd2{hb}")
            nc.sync.dma_start(dd2.rearrange("o (p f) -> (o p) f", p=P), den[:])
            bc = bcp.tile([64, w], F32, tag=f"bc{hb}", name=f"bc{hb}")
            nc.sync.dma_start(bc[:], bass_AP_bcast(dd2, 64, w))
            # multiplies on GPSIMD (SBUF-only operands): keeps DVE free for exp
            if hb == 0:
                nc.gpsimd.tensor_tensor(oT_all[0:64, pr, i0:i1],
                                        oacc[0][0:64, i0:i1], bc[:], OP.mult)
            else:
                tmpb = bcp.tile([64, w], BF16, tag="tmpb", name="tmpb")
                nc.gpsimd.tensor_tensor(tmpb[:], oacc[1][0:64, i0:i1],
                                        bc[:], OP.mult)
                nc.sync.dma_start(oT_all[64:128, pr, i0:i1], tmpb[:])

    # ---- per pair: q/k projection then attention ----
    # Head pairs: head 2pr lives at partitions 0-63 ("A"), 2pr+1 at 64-127
    # ("B") -- their QK matmuls run concurrently in the two row-halves of the
    # PE array.  One slot = one (ib, jc) for both heads: 2 QK matmuls
    # (concurrent), one 1024-wide exp (engines alternate per EXP_PATTERN),
    # 2 AV matmuls.  AV consumption lags TWO slots behind QK/exp so neither
    # exp engine ever stalls the PE; AV accumulates a whole i-block per head
    # in a dedicated PSUM bank, drained once per (ib, head) on ScalarE.
    # Pair 3's normalize runs per-ib as each i-block drains, so the output
    # projection is not gated on a serial end-of-kernel normalize.
    # qkr + avp stay open across all pairs; the q/k projections borrow qkr
    # tiles too (two 8-matmul groups per [P, 2, 512] tile), so PSUM bank
    # reuse at pair boundaries chains against exp reads (done early) rather
    # than a freshly-allocated pool WARing on the final AV drains.
    oacc3 = None
    slot_ctr = 0
    with tc.tile_pool(name="qkr", bufs=3, space="PSUM") as qkr, \
         tc.tile_pool(name="avp", bufs=1, space="PSUM") as avp:
        for pr in range(4):
            for tb in range(TB):
                ps = qkr.tile([P, 2, 512], F32, tag="qk", name="qk")
                for g, eo in enumerate((pr, 4 + pr)):
                    wsb = wq_sb if eo < 4 else wk_sb
                    ee = pr * 128
                    for ko in range(KO):
                        nc.tensor.matmul(ps[:, g, :], wsb[:, ko, ee:ee + 128],
                                         xts[tb][:, ko, :],
                                         start=(ko == 0), stop=(ko == KO - 1))
                    nc.scalar.copy(qkt[eo][tb][:], ps[:, g, :])
            oacc = [asb.tile([65, t], F32, tag="oacc_a", name="oacc_a"),
                    asb.tile([65, t], F32, tag="oacc_b", name="oacc_b")]
            av_tiles = [None, None]

            def flush_av(prev, pr=pr, oacc=oacc, av_tiles=av_tiles, avp=avp):
                es, ib, jc = prev
                for hb in range(2):
                    if jc == 0:
                        av_tiles[hb] = avp.tile([65, 512], F32, tag=f"av{hb}",
                                                name=f"av{hb}")
                    nc.tensor.matmul(av_tiles[hb][:], vsb[:, jc, 2 * pr + hb, :],
                                     es[:, hb, :],
                                     start=(jc == 0), stop=(jc == JC - 1))
                    if jc == JC - 1:
                        nc.scalar.copy(
                            oacc[hb][:, ib * 512:ib * 512 + 512], av_tiles[hb][:])
                if jc == JC - 1 and pr == 3:
                    normalize(3, oacc, ib * 512, ib * 512 + 512)

            # Batches of 3 slots (= qkr bufs): the PE sees runs of 6 QK
            # matmuls then 6 AV matmuls -- same-type back-to-back matmuls
            # pipeline at the 213ns streaming rate, hiding the ~170ns
            # pipeline fill that a QK/AV-alternating stream exposes per
            # transition.
            units = [(ib, jc) for ib in range(IBN) for jc in range(JC)]
            pending = []  # [(es_tile, ib, jc), ...] up to 4-deep
            for b0 in range(0, len(units), 3):
                batch = units[b0:b0 + 3]
                pss = []
                for ib, jc in batch:
                    ps = qkr.tile([P, 2, 512], F32, tag="qk", name="qk")
                    ktile, qtile = qkt[4 + pr][jc // 4], qkt[pr][ib]
                    kk = (jc % 4) * 128
                    for hb in range(2):
                        nc.tensor.matmul(ps[:, hb, :],
                                         ktile[hb * 64:hb * 64 + 64, kk:kk + 128],
                                         qtile[hb * 64:hb * 64 + 64, :],
                                         start=True, stop=True)
                    pss.append(ps)
                for (ib, jc), ps in zip(batch, pss):
                    es = expp.tile([P, 2, 512], BF16, tag="es", name="es")
                    src = ps[:].rearrange("p a b -> p (a b)")
                    dst = es[:].rearrange("p a b -> p (a b)")
                    if EXP_PATTERN[slot_ctr % len(EXP_PATTERN)]:
                        nc.scalar.activation(dst, src, AF.Exp, scale=SCALE)
                    else:
                        nc.vector.tensor_scalar(dst.bitcast(U16), src,
                                                EXP_A, EXP_B, OP.mult, OP.add)
                    slot_ctr += 1
                    pending.append((es, ib, jc))
                while len(pending) > 4:
                    flush_av(pending.pop(0))
            for prev in pending:
                flush_av(prev)
            if pr < 3:
                normalize(pr, oacc, 0, t)
            else:
                oacc3 = oacc

    # ---- output projection (partial over this core's heads) ----
    # Pair 3's oT landed per-ib during its attention, so the c=3
    # accumulations unblock almost immediately.  The first 8 blocks run
    # c=0..2 first (they only need pairs 0-2) as extra slack.  Drain
    # copies alternate DVE/ScalarE (exp work is done here).
    del oacc3
    with tc.tile_pool(name="op_ps", bufs=8, space="PSUM") as opps, \
         tc.tile_pool(name="osb", bufs=4) as osbp:

        def op_block(tci, db, ps, c_lo, c_hi):
            for c in range(c_lo, c_hi):
                nc.tensor.matmul(ps[:], oT_all[:, c, tci * 128:(tci + 1) * 128],
                                 wo_sb[:, c, db * 512:(db + 1) * 512],
                                 start=(c == 0), stop=(c == 3))

        def op_finish(tci, db, ps, k=[0]):
            ot = osbp.tile([P, 512], BF16, tag="ot", name="ot")
            if k[0] % 2 == 0:
                nc.vector.tensor_copy(ot[:], ps[:])
            else:
                nc.scalar.copy(ot[:], ps[:])
            dma_eng = (nc.sync, nc.scalar, nc.gpsimd)[k[0] % 3]
            k[0] += 1
            dma_eng.dma_start(out[tci * 128:(tci + 1) * 128,
                                  db * 512:(db + 1) * 512], ot[:])

        blocks = [(tci, db) for tci in range(TC) for db in range(D // 512)]
        early = []
        for tci, db in blocks[:8]:
            ps = opps.tile([P, 512], F32, tag="op", name="op")
            op_block(tci, db, ps, 0, 3)
            early.append((tci, db, ps))
        for tci, db, ps in early:
            op_block(tci, db, ps, 3, 4)
            op_finish(tci, db, ps)
        for tci, db in blocks[8:]:
            ps = opps.tile([P, 512], F32, tag="op", name="op")
            op_block(tci, db, ps, 0, 4)
            op_finish(tci, db, ps)


def _build(t):
    from contextlib import ExitStack

    nc = bacc.Bacc("TRN2", target_bir_lowering=False, debug=False, num_devices=8)
    # x pre-chunked host-side so each tb DMA is contiguous per partition:
    # xT[tb, p, ko*512+f] = x^T[ko*128+p, tb*512+f]
    xT = nc.dram_tensor("xT", [t // 512, P, KO * 512], BF16,
                        kind="ExternalInput").ap()
    wq = nc.dram_tensor("wq", [P, KO * E], BF16, kind="ExternalInput").ap()
    wk = nc.dram_tensor("wk", [P, KO * E], BF16, kind="ExternalInput").ap()
    wv = nc.dram_tensor("wv", [P, KO * E], BF16, kind="ExternalInput").ap()
    wo = nc.dram_tensor("wo", [E, D], BF16, kind="ExternalInput").ap()
    out = nc.dram_tensor("out", [t, D], BF16, kind="ExternalOutput").ap()
    with tile.TileContext(nc) as tc:
        with ExitStack() as ctx:
            _emit(ctx, tc, nc, xT, wq, wk, wv, wo, out, t)
    nc.compile()
    return nc


def get_compiled(t=T):
    if t not in _cache:
        _cache[t] = _build(t)
    return _cache[t]


def shard_inputs(x, w_qkv, t=T):
    """Per-core input maps (weights reordered head-major, x transposed, bf16)."""
    bf16 = ml_dtypes.bfloat16
    d_idx = np.arange(DH)
    maps = []
    for c in range(8):
        b = c // 2
        heads = np.arange((c % 2) * HL, (c % 2) * HL + HL)
        rows_q = (heads[:, None] + d_idx[None, :] * (3 * H)).reshape(-1)
        rows_k = (heads[:, None] + H + d_idx[None, :] * (3 * H)).reshape(-1)
        rows_v = (heads[:, None] + 2 * H + d_idx[None, :] * (3 * H)).reshape(-1)
        xt = np.ascontiguousarray(x[b][:t].T)
        xt = xt.reshape(KO, 128, t // 512, 512).transpose(2, 1, 0, 3)
        xt = np.ascontiguousarray(xt.reshape(t // 512, 128, KO * 512))

        def wflat(rows):
            # [D, E] -> [P, KO*E]: per-partition contiguous for fast DMA
            w = w_qkv[rows].T.reshape(KO, P, E).transpose(1, 0, 2)
            return np.ascontiguousarray(w.reshape(P, KO * E)).astype(bf16)

        maps.append({
            "xT": xt.astype(bf16),
            "wq": wflat(rows_q),
            "wk": wflat(rows_k),
            "wv": wflat(rows_v),
            "wo": None,  # filled below
        })
    return maps


def kernel(x, w_qkv, w_out, b_out):
    x = np.asarray(x, dtype=np.float32)
    w_qkv = np.asarray(w_qkv, dtype=np.float32)
    w_out = np.asarray(w_out, dtype=np.float32)
    b_out = np.asarray(b_out, dtype=np.float32)

    nc = get_compiled(T)
    d_idx = np.arange(DH)
    in_maps = shard_inputs(x, w_qkv, T)
    for c in range(8):
        heads = np.arange((c % 2) * HL, (c % 2) * HL + HL)
        cols_o = (heads[:, None] * DH + d_idx[None, :]).reshape(-1)
        in_maps[c]["wo"] = np.ascontiguousarray(
            w_out[:, cols_o].T).astype(ml_dtypes.bfloat16)

    res = run_bass_kernel_spmd(nc, in_maps, core_ids=list(range(8)))
    global last_results
    last_results = res

    out = np.empty((B, T, D), dtype=np.float32)
    for b in range(B):
        out[b] = (res.results[2 * b]["out"].astype(np.float32) +
                  res.results[2 * b + 1]["out"].astype(np.float32))
    out += b_out
    return out.reshape(B, T, D)


# revision 58
# speedup vs baseline: 1.0270x; 1.0270x over previous
"""Multi-head attention (B=4, T=2048, D=1024, H=16, DH=64) on 8 Trainium2 NeuronCores.

Sharding (data + tensor parallel, no collectives):
  core c owns batch b = c//2 and heads [(c%2)*8, (c%2)*8 + 8).
  Each core computes q/k/v projections for its batch over its 8 heads, the
  full attention for those (batch, head) slabs, and a partial output
  projection over its heads' columns of w_out.  The host sums the two
  partial outputs per batch (the only cross-core reduction).

Key optimizations vs a straightforward fp32r implementation (~477us -> ~355us
measured; the major wins):
  - All matmul operands are bf16 (fp32 moving operands stream at 2 cyc/row on
    the PE; bf16 streams at 1 -> 2x matmul throughput).  PSUM stays fp32.
  - The softmax exp (33.5M elements/core, a ScalarE bottleneck at 1
    elem/lane/cycle) alternates between ScalarE (exact exp) and the DVE
    (Schraudolph-style bit-trick exp emitting bf16 bit patterns via a single
    tensor_scalar mult+add with uint16 convert).  The softmax is
    self-normalizing, so the ~2% sawtooth error of the bit-trick on half the
    j-positions washes out to ~1e-2 relative error in the output.
  - Attention runs as 3-slot batches (slot = one (i-block, j-chunk) for a
    head pair): the PE sees runs of 6 QK then 6 AV matmuls -- same-type
    back-to-back matmuls pipeline at the 213ns streaming rate while
    alternating types exposes a ~170ns pipeline fill each transition.  AV
    consumption lags exp by 4+ slots so neither exp engine stalls the PE.
  - q/k projections are interleaved per head-pair with that pair's
    attention; softmax normalization runs per-i-block as AV blocks drain
    (GPSIMD does the multiplies), so the output projection is never gated
    on a serial end-of-kernel normalize.

Device algorithm (per core), everything feature-major ("transposed") so the
contraction dim always lands on SBUF partitions:
  qT = Wq @ x^T            [512, T]   (rows = h_local*64 + d)
  kT = Wk @ x^T            [512, T]
  V  = x @ Wv^T            [T, 512]   (+ a ones column per head => denominator)
  per head h: ST = kT_h^T-contraction  => scores [j, i] in PSUM,
              E = exp(ST/32) (ScalarE fused scale, or DVE bit-trick),
              oT_h' = [V_h | 1]^T @ E  => [65, i] (row 64 = softmax denom),
              oT_h = oT_h[0:64] * (1/denom) broadcast (DRAM-bounce bcast + DVE)
  out_partial = oT^T @ Wo_slice^T  [T, 1024]
"""

import os
import sys

import numpy as np
import ml_dtypes

if "/opt/trn_rl_repo" not in sys.path and os.path.isdir("/opt/trn_rl_repo"):
    sys.path.insert(0, "/opt/trn_rl_repo")

import concourse.bass as bass  # noqa: E402
import concourse.tile as tile  # noqa: E402
from concourse import bacc, mybir  # noqa: E402
from concourse.bass_utils import run_bass_kernel_spmd  # noqa: E402


def bass_AP_bcast(tile_ap, parts, free):
    """Partition-broadcast AP over a DRAM [1, free] tile (step-0 partition)."""
    a = tile_ap[:]
    return bass.AP(tensor=a.tensor, offset=a.offset,
                   ap=[[0, parts]] + [list(x) for x in a.ap[1:]])

F32 = mybir.dt.float32
BF16 = mybir.dt.bfloat16
U16 = mybir.dt.uint16
AF = mybir.ActivationFunctionType
OP = mybir.AluOpType

B, T, D, H, DH = 4, 2048, 1024, 16, 64
HL = H // 2          # heads per core
E = HL * DH          # 512: per-core q (or k, or v) feature width
KO = D // 128        # 8 contraction chunks for the projections
P = 128
SCALE = float(D) ** -0.5  # note: dim**-0.5, faithful to the reference

# Schraudolph exp -> bf16 bits: bf16(exp(SCALE*s)) ~= u16(EXP_A*s+EXP_B).
# A = SCALE*log2(e)*2^7 maps the exponent into the bf16 exponent field;
# B centers at bias 127 with a -5.50/128 minimax offset balancing the
# (1+f)/2^f mantissa-linear approximation, +0.5 assuming truncating convert
# (if the hw rounds instead, the uniform x2^(0.5/128) factor cancels in the
# softmax).
EXP_A = SCALE * 1.4426950408889634 * 128.0
EXP_B = 127.0 * 128.0 - 5.50 + 0.5

# exp engine schedule: per attention slot, cycle through this pattern.
# True = ScalarE exact exp, False = DVE bit-trick.
EXP_PATTERN = (True, False)  # alternate ACT / DVE

_cache = {}
last_results = None


def _emit(ctx, tc, nc, xT, wq, wk, wv, wo, out, t):
    TB = t // 512        # moving-dim blocks for projections
    TC = t // 128        # t chunks (also j chunks)
    JC = t // 128
    IBN = t // 512       # query i-blocks

    xT3 = xT.rearrange("tb p (ko f) -> tb p ko f", ko=KO)
    wq3 = wq.rearrange("p (ko e) -> p ko e", ko=KO)
    wk3 = wk.rearrange("p (ko e) -> p ko e", ko=KO)
    wv3 = wv.rearrange("p (ko e) -> p ko e", ko=KO)
    wo3 = wo.rearrange("(c p) d -> p c d", p=P)

    persist = ctx.enter_context(tc.tile_pool(name="persist", bufs=1))
    # q/k tiles split per (eo, tb) so attention slots dep only on the copies
    # they actually read (tile-granular tracking would gate each pair's first
    # QK on the final projection copy).  eo 0-3: q pairs, 4-7: k pairs.
    qkt = [[persist.tile([P, 512], BF16, tag=f"qkT{eo}_{tb}",
                         name=f"qkT{eo}_{tb}")
            for tb in range(TB)] for eo in range(8)]
    # V is zero-padded from 65 (64 dims + ones/denominator column) to 128
    # columns so the AV LDWEIGHTS is a full-128-col bf16 weight and qualifies
    # for fast weight load; the pad columns yield zero PSUM rows the drains
    # never read.  The pad memset runs on GPSIMD (idle at startup) so it does
    # not delay the V-projection drain copies on the DVE.
    vsb = persist.tile([P, JC, HL, 128], BF16, tag="v")
    ones1 = persist.tile([P, 1], BF16, tag="ones")
    nc.vector.memset(ones1[:], 1.0)
    nc.gpsimd.memset(vsb[:, :, :, DH + 1:128], 0.0)
    nc.vector.tensor_copy(vsb[:, :, :, DH], ones1.to_broadcast((P, JC, HL)))
    oT_all = persist.tile([P, 4, t], BF16, tag="oT")

    wp = ctx.enter_context(tc.tile_pool(name="w_p", bufs=1))
    wv_sb = wp.tile([P, KO, E], BF16, tag="wv")
    wq_sb = wp.tile([P, KO, E], BF16, tag="wq")
    wk_sb = wp.tile([P, KO, E], BF16, tag="wk")
    wo_sb = wp.tile([P, 4, D], BF16, tag="wo")
    xts = [wp.tile([P, KO, 512], BF16, tag=f"xt{tb}", name=f"xt{tb}")
           for tb in range(TB)]
    # Inputs split across the two hwdge queues (sync + scalar) so x and the
    # weights stream in parallel; gpsimd queue takes the non-urgent wo.  The
    # first V-proj matmul group needs xt0[ko] + wv[ko] in ko order, so those
    # two stream as ko-pair chunks on separate queues and the projection
    # starts ~2us after the engine-sync preamble instead of waiting for
    # whole-tensor DMAs.
    for c in range(KO // 2):
        nc.sync.dma_start(xts[0][:, 2 * c:2 * c + 2, :],
                          xT3[0][:, 2 * c:2 * c + 2, :])
        nc.scalar.dma_start(wv_sb[:, 2 * c:2 * c + 2, :],
                            wv3[:, 2 * c:2 * c + 2, :])
    nc.sync.dma_start(xts[1][:], xT3[1])
    nc.sync.dma_start(xts[2][:], xT3[2])
    nc.scalar.dma_start(xts[3][:], xT3[3])
    nc.scalar.dma_start(wq_sb[:], wq3)
    nc.scalar.dma_start(wk_sb[:], wk3)
    nc.gpsimd.dma_start(wo_sb[:], wo3)

    # ---- V projection (needed by every pair's AV) ----
    with tc.tile_pool(name="vps", bufs=4, space="PSUM") as vpsp:
        for tci in range(TC):
            xt = xts[tci // 4]
            sub = tci % 4
            ps = vpsp.tile([P, 512], F32, tag="vps")
            for ko in range(KO):
                nc.tensor.matmul(ps[:], xt[:, ko, sub * 128:(sub + 1) * 128],
                                 wv_sb[:, ko, :],
                                 start=(ko == 0), stop=(ko == KO - 1))
            nc.vector.tensor_copy(vsb[:, tci, :, 0:DH],
                                  ps.rearrange("p (h d) -> p h d", d=DH))

    # shared SBUF pools for attention
    asb = ctx.enter_context(tc.tile_pool(name="attn", bufs=1))
    expp = ctx.enter_context(tc.tile_pool(name="expp", bufs=8))
    bcp = ctx.enter_context(tc.tile_pool(name="bc", bufs=1))
    dramp = ctx.enter_context(tc.tile_pool(name="dram", bufs=2, space="DRAM"))

    def normalize(pr, oacc, i0, i1):
        # oT[:, pr, i0:i1] = oacc[0:64, i0:i1] * (1 / denom), denom = row 64.
        # The denom row sits on ONE partition; bounce it through DRAM to
        # reshape to [128, w/128] (reciprocal on 128 lanes), back to DRAM,
        # then DMA-broadcast from DRAM across 64 partitions.  The B-half
        # product needs a partition shift to rows 64-127: SBUF->SBUF DMA.
        # hb=0 and hb=1 chains are independent (distinct tags) and overlap.
        w = i1 - i0
        for hb in range(2):
            dd1 = dramp.tile([1, w], F32, tag=f"dd1{hb}", name=f"dd1{hb}")
            nc.sync.dma_start(dd1[:], oacc[hb][64:65, i0:i1])
            den = bcp.tile([P, w // P], F32, tag=f"den{hb}", name=f"den{hb}")
            nc.sync.dma_start(den[:], dd1.rearrange("o (p f) -> (o p) f", p=P))
            nc.vector.reciprocal(den[:], den[:])
            dd2 = dramp.tile([1, w], F32, tag=f"dd2{hb}", name=f"dd2{hb}")
            nc.sync.dma_start(dd2.rearrange("o (p f) -> (o p) f", p=P), den[:])
            bc = bcp.tile([64, w], F32, tag=f"bc{hb}", name=f"bc{hb}")
            nc.sync.dma_start(bc[:], bass_AP_bcast(dd2, 64, w))
            # multiplies on GPSIMD (SBUF-only operands): keeps DVE free for exp
            if hb == 0:
                nc.gpsimd.tensor_tensor(oT_all[0:64, pr, i0:i1],
                                        oacc[0][0:64, i0:i1], bc[:], OP.mult)
            else:
                tmpb = bcp.tile([64, w], BF16, tag="tmpb", name="tmpb")
                nc.gpsimd.tensor_tensor(tmpb[:], oacc[1][0:64, i0:i1],
                                        bc[:], OP.mult)
                nc.sync.dma_start(oT_all[64:128, pr, i0:i1], tmpb[:])

    # ---- per pair: q/k projection then attention ----
    # Head pairs: head 2pr lives at partitions 0-63 ("A"), 2pr+1 at 64-127
    # ("B") -- their QK matmuls run concurrently in the two row-halves of the
    # PE array.  One slot = one (ib, jc) for both heads: 2 QK matmuls
    # (concurrent), one 1024-wide exp (engines alternate per EXP_PATTERN),
    # 2 AV matmuls.  AV consumption lags TWO slots behind QK/exp so neither
    # exp engine ever stalls the PE; AV accumulates a whole i-block per head
    # in a dedicated PSUM bank, drained once per (ib, head) on ScalarE.
    # Pair 3's normalize runs per-ib as each i-block drains, so the output
    # projection is not gated on a serial end-of-kernel normalize.
    # qkr + avp stay open across all pairs; the q/k projections borrow qkr
    # tiles too (two 8-matmul groups per [P, 2, 512] tile), so PSUM bank
    # reuse at pair boundaries chains against exp reads (done early) rather
    # than a freshly-allocated pool WARing on the final AV drains.
    oacc3 = None
    slot_ctr = 0
    with tc.tile_pool(name="qkr", bufs=3, space="PSUM") as qkr, \
         tc.tile_pool(name="avp", bufs=1, space="PSUM") as avp:
        for pr in range(4):
            for tb in range(TB):
                ps = qkr.tile([P, 2, 512], F32, tag="qk", name="qk")
                for g, eo in enumerate((pr, 4 + pr)):
                    wsb = wq_sb if eo < 4 else wk_sb
                    ee = pr * 128
                    for ko in range(KO):
                        nc.tensor.matmul(ps[:, g, :], wsb[:, ko, ee:ee + 128],
                                         xts[tb][:, ko, :],
                                         start=(ko == 0), stop=(ko == KO - 1))
                    nc.scalar.copy(qkt[eo][tb][:], ps[:, g, :])
            oacc = [asb.tile([65, t], F32, tag="oacc_a", name="oacc_a"),
                    asb.tile([65, t], F32, tag="oacc_b", name="oacc_b")]
            av_tiles = [None, None]

            def flush_av(prev, pr=pr, oacc=oacc, av_tiles=av_tiles, avp=avp):
                es, ib, jc = prev
                for hb in range(2):
                    if jc == 0:
                        av_tiles[hb] = avp.tile([P, 512], F32, tag=f"av{hb}",
                                                name=f"av{hb}")
                    nc.tensor.matmul(av_tiles[hb][:], vsb[:, jc, 2 * pr + hb, :],
                                     es[:, hb, :],
                                     start=(jc == 0), stop=(jc == JC - 1))
                    if jc == JC - 1:
                        nc.scalar.copy(oacc[hb][:, ib * 512:ib * 512 + 512],
                                       av_tiles[hb][0:65, :])
                if jc == JC - 1 and pr == 3:
                    normalize(3, oacc, ib * 512, ib * 512 + 512)

            # Batches of 3 slots (= qkr bufs): the PE sees runs of 6 QK
            # matmuls then 6 AV matmuls -- same-type back-to-back matmuls
            # pipeline at the 213ns streaming rate, hiding the ~170ns
            # pipeline fill that a QK/AV-alternating stream exposes per
            # transition.
            units = [(ib, jc) for ib in range(IBN) for jc in range(JC)]
            pending = []  # [(es_tile, ib, jc), ...] up to 4-deep
            for b0 in range(0, len(units), 3):
                batch = units[b0:b0 + 3]
                pss = []
                for ib, jc in batch:
                    ps = qkr.tile([P, 2, 512], F32, tag="qk", name="qk")
                    ktile, qtile = qkt[4 + pr][jc // 4], qkt[pr][ib]
                    kk = (jc % 4) * 128
                    for hb in range(2):
                        nc.tensor.matmul(ps[:, hb, :],
                                         ktile[hb * 64:hb * 64 + 64, kk:kk + 128],
                                         qtile[hb * 64:hb * 64 + 64, :],
                                         start=True, stop=True)
                    pss.append(ps)
                for (ib, jc), ps in zip(batch, pss):
                    es = expp.tile([P, 2, 512], BF16, tag="es", name="es")
                    src = ps[:].rearrange("p a b -> p (a b)")
                    dst = es[:].rearrange("p a b -> p (a b)")
                    if EXP_PATTERN[slot_ctr % len(EXP_PATTERN)]:
                        nc.scalar.activation(dst, src, AF.Exp, scale=SCALE)
                    else:
                        nc.vector.tensor_scalar(dst.bitcast(U16), src,
                                                EXP_A, EXP_B, OP.mult, OP.add)
                    slot_ctr += 1
                    pending.append((es, ib, jc))
                while len(pending) > 4:
                    flush_av(pending.pop(0))
            for prev in pending:
                flush_av(prev)
            if pr < 3:
                normalize(pr, oacc, 0, t)
            else:
                oacc3 = oacc

    # ---- output projection (partial over this core's heads) ----
    # Pair 3's oT landed per-ib during its attention, so the c=3
    # accumulations unblock almost immediately.  The first 8 blocks run
    # c=0..2 first (they only need pairs 0-2) as extra slack.  Drain
    # copies alternate DVE/ScalarE (exp work is done here).
    del oacc3
    with tc.tile_pool(name="op_ps", bufs=8, space="PSUM") as opps, \
         tc.tile_pool(name="osb", bufs=4) as osbp:

        def op_block(tci, db, ps, c_lo, c_hi):
            for c in range(c_lo, c_hi):
                nc.tensor.matmul(ps[:], oT_all[:, c, tci * 128:(tci + 1) * 128],
                                 wo_sb[:, c, db * 512:(db + 1) * 512],
                                 start=(c == 0), stop=(c == 3))

        def op_finish(tci, db, ps, k=[0]):
            ot = osbp.tile([P, 512], BF16, tag="ot", name="ot")
            if k[0] % 2 == 0:
                nc.vector.tensor_copy(ot[:], ps[:])
            else:
                nc.scalar.copy(ot[:], ps[:])
            dma_eng = (nc.sync, nc.scalar, nc.gpsimd)[k[0] % 3]
            k[0] += 1
            dma_eng.dma_start(out[tci * 128:(tci + 1) * 128,
                                  db * 512:(db + 1) * 512], ot[:])

        blocks = [(tci, db) for tci in range(TC) for db in range(D // 512)]
        early = []
        for tci, db in blocks[:8]:
            ps = opps.tile([P, 512], F32, tag="op", name="op")
            op_block(tci, db, ps, 0, 3)
            early.append((tci, db, ps))
        for tci, db, ps in early:
            op_block(tci, db, ps, 3, 4)
            op_finish(tci, db, ps)
        for tci, db in blocks[8:]:
            ps = opps.tile([P, 512], F32, tag="op", name="op")
            op_block(tci, db, ps, 0, 4)
            op_finish(tci, db, ps)


def _build(t):
    from contextlib import ExitStack

    nc = bacc.Bacc("TRN2", target_bir_lowering=False, debug=False, num_devices=8)
    # x pre-chunked host-side so each tb DMA is contiguous per partition:
    # xT[tb, p, ko*512+f] = x^T[ko*128+p, tb*512+f]
    xT = nc.dram_tensor("xT", [t // 512, P, KO * 512], BF16,
                        kind="ExternalInput").ap()
    wq = nc.dram_tensor("wq", [P, KO * E], BF16, kind="ExternalInput").ap()
    wk = nc.dram_tensor("wk", [P, KO * E], BF16, kind="ExternalInput").ap()
    wv = nc.dram_tensor("wv", [P, KO * E], BF16, kind="ExternalInput").ap()
    wo = nc.dram_tensor("wo", [E, D], BF16, kind="ExternalInput").ap()
    out = nc.dram_tensor("out", [t, D], BF16, kind="ExternalOutput").ap()
    with tile.TileContext(nc) as tc:
        with ExitStack() as ctx:
            _emit(ctx, tc, nc, xT, wq, wk, wv, wo, out, t)
    nc.compile()
    return nc


def get_compiled(t=T):
    if t not in _cache:
        _cache[t] = _build(t)
    return _cache[t]


def shard_inputs(x, w_qkv, t=T):
    """Per-core input maps (weights reordered head-major, x transposed, bf16)."""
    bf16 = ml_dtypes.bfloat16
    d_idx = np.arange(DH)
    maps = []
    for c in range(8):
        b = c // 2
        heads = np.arange((c % 2) * HL, (c % 2) * HL + HL)
        rows_q = (heads[:, None] + d_idx[None, :] * (3 * H)).reshape(-1)
        rows_k = (heads[:, None] + H + d_idx[None, :] * (3 * H)).reshape(-1)
        rows_v = (heads[:, None] + 2 * H + d_idx[None, :] * (3 * H)).reshape(-1)
        xt = np.ascontiguousarray(x[b][:t].T)
        xt = xt.reshape(KO, 128, t // 512, 512).transpose(2, 1, 0, 3)
        xt = np.ascontiguousarray(xt.reshape(t // 512, 128, KO * 512))

        def wflat(rows):
            # [D, E] -> [P, KO*E]: per-partition contiguous for fast DMA
            w = w_qkv[rows].T.reshape(KO, P, E).transpose(1, 0, 2)
            return np.ascontiguousarray(w.reshape(P, KO * E)).astype(bf16)

        maps.append({
            "xT": xt.astype(bf16),
            "wq": wflat(rows_q),
            "wk": wflat(rows_k),
            "wv": wflat(rows_v),
            "wo": None,  # filled below
        })
    return maps


def kernel(x, w_qkv, w_out, b_out):
    x = np.asarray(x, dtype=np.float32)
    w_qkv = np.asarray(w_qkv, dtype=np.float32)
    w_out = np.asarray(w_out, dtype=np.float32)
    b_out = np.asarray(b_out, dtype=np.float32)

    nc = get_compiled(T)
    d_idx = np.arange(DH)
    in_maps = shard_inputs(x, w_qkv, T)
    for c in range(8):
        heads = np.arange((c % 2) * HL, (c % 2) * HL + HL)
        cols_o = (heads[:, None] * DH + d_idx[None, :]).reshape(-1)
        in_maps[c]["wo"] = np.ascontiguousarray(
            w_out[:, cols_o].T).astype(ml_dtypes.bfloat16)

    res = run_bass_kernel_spmd(nc, in_maps, core_ids=list(range(8)))
    global last_results
    last_results = res

    out = np.empty((B, T, D), dtype=np.float32)
    for b in range(B):
        out[b] = (res.results[2 * b]["out"].astype(np.float32) +
                  res.results[2 * b + 1]["out"].astype(np.float32))
    out += b_out
    return out.reshape(B, T, D)
